# revision 13
# baseline (speedup 1.0000x reference)
"""Trainium2 Bass kernel for nn_LunarisCodex (2-layer Switch-MoE transformer).

8 NeuronCores, token-sharded trunk (256 tokens/core), expert-parallel MoE
(1 expert/core/layer), vocab-sharded lm_head (6283 cols/core).

Collectives per layer: AllGather of rope'd kT/v within batch groups
[[0-3],[4-7]]; global AllGathers of gate logits (tiny), ffn_in (dispatch),
and expert outputs (combine). Final: global AllGather of normed activations
(bf16, transposed) for the lm_head.

Routing (argmax, capacity ranks via strict-triangular matmuls, aux losses) is
computed replicated on every core from the gathered gate logits; dispatch and
combine are local indirect-DMA gathers keyed by device-computed index maps.

Precision: fp32r (full-rate ~13-bit-mantissa PE mode) for every matmul that
feeds routing (attention both layers, layer-0 experts); bf16 for layer-1
experts and lm_head; fp32 for gate logits, softmax, norms. This reproduces
the fp32 reference's expert assignments exactly (margin analysis: worst
perturbation/margin = 0.33).
"""

import math
import sys

import numpy as np

for _p in ("/opt/trn_rl_repo", "/root/.axon_site/_ro/trn_rl_repo"):
    if _p not in sys.path:
        sys.path.insert(0, _p)

import concourse.bacc as bacc
import concourse.bass as bass
import concourse.mybir as mybir
import concourse.tile as tile
from concourse.bass_utils import run_bass_kernel_spmd

F32 = mybir.dt.float32
F32R = mybir.dt.float32r
BF16 = mybir.dt.bfloat16
I32 = mybir.dt.int32
AF = mybir.ActivationFunctionType
ALU = mybir.AluOpType
AX = mybir.AxisListType

B, T, D, NH, HD = 2, 1024, 768, 12, 64
V, L, E, H = 50257, 2, 8, 2048
N = B * T
CAP = 320
AUX_W, Z_W, THETA, EPS = 0.01, 0.001, 10000.0, 1e-5
NC = 8
TC = N // NC          # 256 tokens/core
MT = TC // 128        # 2
DK = D // 128         # 6
HK = H // 128         # 16
VP = 6283             # padded vocab cols/core
GRP_BATCH = [[0, 1, 2, 3], [4, 5, 6, 7]]
GRP_ALL = [list(range(NC))]


def build_program(stage="full"):
    nc = bacc.Bacc(None, target_bir_lowering=False, num_devices=NC)

    def inp(name, shape, dt=F32):
        return nc.dram_tensor(name, shape, dt, kind="ExternalInput")

    x0 = inp("x0", [TC, D])
    cos_t = inp("cos_t", [TC, HD // 2])
    sin_t = inp("sin_t", [TC, HD // 2])
    cmask = inp("cmask", [1024, TC])
    qnw = inp("qnw", [L, 128, D])
    knw = inp("knw", [L, 128, D])
    fnw = inp("fnw", [L, 128, D])
    lnf = inp("lnf", [128, D])
    wqkv = inp("wqkv", [L, D, 3 * D], F32R)
    wop = inp("wop", [L, D, D], F32R)
    gatew = inp("gatew", [L, D, E])
    w13_0 = inp("w13_0", [D, 2 * H], F32R)
    w2_0 = inp("w2_0", [H, D], F32R)
    w13_1 = inp("w13_1", [D, 2 * H], BF16)
    w2_1 = inp("w2_1", [H, D], BF16)
    wteT = inp("wteT", [D, VP], BF16)
    ident = inp("ident", [128, 128])
    triu = inp("triu", [128, 128])
    onescol = inp("onescol", [128, 1])
    ones1r = inp("ones1r", [1, 128])
    eiota = inp("eiota", [128, E])
    myexp = inp("myexp", [128, 1])
    tokid_all = inp("tokid_all", [N, 1], I32)
    mytok = inp("mytok", [128, MT], I32)

    logits_o = nc.dram_tensor("logits", [N, VP], F32, kind="ExternalOutput")
    aux_o = nc.dram_tensor("aux", [1, 1], F32, kind="ExternalOutput")
    dbg = {}
    if stage != "full":
        for nm, shp in [("h", [TC, D]), ("f", [TC, D]), ("top", [N, 1]),
                        ("rank", [N, 1]), ("xe", [CAP, D]), ("ye", [CAP, D]),
                        ("x1", [TC, D])]:
            dbg[nm] = nc.dram_tensor("dbg_" + nm, shp, F32, kind="ExternalOutput")

    with tile.TileContext(nc) as tc:
        with tc.tile_pool(name="sb", bufs=1) as sb, \
             tc.tile_pool(name="sc", bufs=2) as sc, \
             tc.tile_pool(name="wp", bufs=3) as wp, \
             tc.tile_pool(name="pp", bufs=8, space="PSUM") as pp, \
             tc.tile_pool(name="dram", bufs=1, space="DRAM") as dram:

            kv_in = dram.tile([2, D * TC], F32R)
            kv_out = dram.tile([4, 2, D * TC], F32R)
            lg_in = dram.tile([TC, E], F32)
            lg_outs = []
            for _li in range(L):
                lg_out_l = dram.tile([N, E], F32, addr_space="Shared")
                lg_outs.append(lg_out_l)
            f_in = dram.tile([TC, D], F32)
            f_outs = []
            for _li in range(L):
                f_out_l = dram.tile([N, D], F32, addr_space="Shared")
                f_outs.append(f_out_l)
            ye_in0 = dram.tile([CAP, D], F32)
            ye_out0 = dram.tile([NC * CAP, D], F32, addr_space="Shared")
            ye_in1 = dram.tile([CAP, D], BF16)
            ye_out1 = dram.tile([NC * CAP, D], BF16, addr_space="Shared")
            xt_in = dram.tile([D * TC], BF16)
            xt_out = dram.tile([NC, D * TC], BF16, addr_space="Shared")
            tmap = dram.tile([CAP + 1, 1], I32)
            meta = dram.tile([N, 2], F32)

            def const(name_tag, src_ap, shape, dt=F32):
                t = sb.tile(shape, dt, tag=name_tag)
                nc.sync.dma_start(out=t[:], in_=src_ap)
                return t

            ident_t = const("ident", ident[:], [128, 128])
            triu_t = const("triu", triu[:], [128, 128])
            onescol_t = const("onescol", onescol[:], [128, 1])
            ones1r_t = const("ones1r", ones1r[:], [1, 128])
            eiota_t = const("eiota", eiota[:], [128, E])
            myexp_t = const("myexp", myexp[:], [128, 1])
            cos_sb = const("cos", cos_t[:].rearrange("(m p) i -> p m i", p=128),
                           [128, MT, HD // 2])
            sin_sb = const("sin", sin_t[:].rearrange("(m p) i -> p m i", p=128),
                           [128, MT, HD // 2])
            cmask_sb = const("cmask", cmask[:].rearrange("(a p) q -> p a q", p=128),
                             [128, 8, TC])
            tok_sb = const("tok", tokid_all[:].rearrange("(a p) one -> p (a one)", p=128),
                           [128, 16], I32)
            mytok_sb = const("mytok", mytok[:], [128, MT], I32)
            onescol_r = sb.tile([128, 1], F32R, tag="onescolr")
            nc.vector.tensor_copy(out=onescol_r[:], in_=onescol_t[:])
            ones1r_r = sb.tile([1, 128], F32R, tag="ones1rr")
            nc.vector.tensor_copy(out=ones1r_r[:], in_=ones1r_t[:])

            eps_t = sb.tile([128, 1], F32, tag="eps")
            nc.vector.memset(eps_t[:], EPS)
            x_sb = sb.tile([128, MT, D], F32, tag="x")
            nc.sync.dma_start(out=x_sb[:], in_=x0[:].rearrange("(m p) d -> p m d", p=128))
            aux_sb = sb.tile([1, 1], F32, tag="aux")
            nc.vector.memset(aux_sb[:], 0.0)

            def pe_t(dst, src):
                """PE transpose one tile: src (p<=128, f<=128) -> dst (f, p)."""
                ps = pp.tile([128, 128], F32, tag="ps", space="PSUM")
                pn = src.partition_size()
                nc.tensor.transpose(out=ps[:src.shape[-1], :pn],
                                    in_=src, identity=ident_t[:pn, :pn])
                nc.vector.tensor_copy(out=dst, in_=ps[:dst.partition_size(), :dst.shape[-1]])

            def transpose_mtD(src, dst, mts=MT):
                for m in range(mts):
                    for dk in range(DK):
                        pe_t(dst[:, dk, m * 128:(m + 1) * 128],
                             src[:, m, dk * 128:(dk + 1) * 128])

            def rmsnorm_tiles(src, dst, w_dram, tg):
                w_t = sc.tile([128, D], F32, tag="nw")
                nc.sync.dma_start(out=w_t[:], in_=w_dram)
                for m in range(MT):
                    sq = sc.tile([128, D], F32, tag="nsq")
                    ssum = sc.tile([128, 1], F32, tag="nss")
                    nc.scalar.activation(sq[:], src[:, m, :], AF.Square, accum_out=ssum[:])
                    rms = sc.tile([128, 1], F32, tag="nrm")
                    nc.scalar.activation(rms[:], ssum[:], AF.Sqrt, bias=eps_t[:], scale=1.0 / D)
                    rinv = sc.tile([128, 1], F32, tag="nri")
                    nc.vector.reciprocal(out=rinv[:], in_=rms[:])
                    nc.vector.tensor_scalar_mul(dst[:, m, :], src[:, m, :], rinv[:])
                    nc.vector.tensor_mul(dst[:, m, :], dst[:, m, :], w_t[:])

            h_sb = sb.tile([128, MT, D], F32, tag="hres")

            for l in range(L):
                lg_out = lg_outs[l]
                f_out = f_outs[l]
                # ================= attention =================
                xT = sb.tile([128, DK, TC], F32R, tag="dmaj")
                transpose_mtD(x_sb, xT)
                q_sb = sb.tile([128, MT, D], F32, tag="t6a")
                k_sb = sb.tile([128, MT, D], F32, tag="t6b")
                v_sb = sb.tile([128, MT, D], F32R, tag="t6c")
                qkv_dst = [(q_sb, 0), (k_sb, D), (v_sb, 2 * D)]
                nch = [(i * 512, min(512, 3 * D - i * 512)) for i in range((3 * D + 511) // 512)]
                for n0, nw in nch:
                    wt = wp.tile([128, DK, 512], F32R, tag="w")
                    for dk in range(DK):
                        nc.sync.dma_start(out=wt[:, dk, :nw],
                                          in_=wqkv[l, dk * 128:(dk + 1) * 128, n0:n0 + nw])
                    for m in range(MT):
                        ps = pp.tile([128, 512], F32, tag="ps", space="PSUM")
                        for dk in range(DK):
                            nc.tensor.matmul(out=ps[:, :nw],
                                             lhsT=xT[:, dk, m * 128:(m + 1) * 128],
                                             rhs=wt[:, dk, :nw],
                                             start=(dk == 0), stop=(dk == DK - 1))
                        for dst, base in qkv_dst:
                            lo, hi = max(n0, base), min(n0 + nw, base + D)
                            if lo < hi:
                                nc.vector.tensor_copy(out=dst[:, m, lo - base:hi - base],
                                                      in_=ps[:, lo - n0:hi - n0])
                rmsnorm_tiles(q_sb, q_sb, qnw[l], "qn")
                rmsnorm_tiles(k_sb, k_sb, knw[l], "kn")
                for src in (q_sb, k_sb):
                    for m in range(MT):
                        ap = src[:, m, :].rearrange("p (h i two) -> p h i two",
                                                    i=HD // 2, two=2)
                        e0, e1 = ap[:, :, :, 0], ap[:, :, :, 1]
                        cs = cos_sb[:, m, :].unsqueeze(1).broadcast_to([128, NH, HD // 2])
                        sn = sin_sb[:, m, :].unsqueeze(1).broadcast_to([128, NH, HD // 2])
                        t0 = sc.tile([128, NH, HD // 2], F32, tag="ra")
                        t1 = sc.tile([128, NH, HD // 2], F32, tag="rb")
                        t2 = sc.tile([128, NH, HD // 2], F32, tag="rc")
                        nc.vector.tensor_mul(t0[:], e0, cs)
                        nc.vector.tensor_mul(t1[:], e1, sn)
                        nc.vector.tensor_mul(t2[:], e0, sn)
                        nc.vector.tensor_mul(e1, e1, cs)
                        nc.vector.tensor_add(e1, e1, t2[:])
                        nc.vector.tensor_sub(e0, t0[:], t1[:])

                kT = sb.tile([128, DK, TC], F32R, tag="dmaj2")
                transpose_mtD(k_sb, kT)
                nc.sync.dma_start(out=kv_in[0].rearrange("(p x) -> p x", p=128),
                                  in_=kT[:].rearrange("p a x -> p (a x)"))
                nc.sync.dma_start(out=kv_in[1].rearrange("(p x) -> p x", p=128),
                                  in_=v_sb[:].rearrange("p m d -> p (m d)"))
                nc.gpsimd.collective_compute("AllGather", ALU.bypass,
                                             replica_groups=GRP_BATCH,
                                             ins=[kv_in[:].opt()], outs=[kv_out[:].opt()])
                kTa = sb.tile([128, 4 * DK, TC], F32R, tag="bigA")
                va = sb.tile([128, 4 * MT, D], F32R, tag="bigB")
                for cb in range(4):
                    nc.sync.dma_start(
                        out=kTa[:, cb * DK:(cb + 1) * DK, :].rearrange("p a x -> p (a x)"),
                        in_=kv_out[cb, 0].rearrange("(p x) -> p x", p=128))
                    nc.sync.dma_start(
                        out=va[:, cb * MT:(cb + 1) * MT, :].rearrange("p m d -> p (m d)"),
                        in_=kv_out[cb, 1].rearrange("(p x) -> p x", p=128))

                qT = sb.tile([128, DK, TC], F32R, tag="dmaj")
                transpose_mtD(q_sb, qT)

                yT = sb.tile([128, DK, TC], F32R, tag="yT")
                isq = 1.0 / math.sqrt(HD)
                for h in range(NH):
                    dk_h, off = h // 2, (h % 2) * 64
                    sums_ps = pp.tile([1, TC], F32, tag="ps", space="PSUM")
                    av_ps = pp.tile([64, TC], F32, tag="ps", space="PSUM")
                    for tk in range(8):
                        cb, j = tk // 2, tk % 2
                        s_ps = pp.tile([128, TC], F32, tag="ps", space="PSUM")
                        nc.tensor.matmul(
                            out=s_ps[:],
                            lhsT=kTa[off:off + 64, cb * DK + dk_h,
                                     j * 128:(j + 1) * 128],
                            rhs=qT[off:off + 64, dk_h, :],
                            start=True, stop=True)
                        ex = sc.tile([128, TC], F32R, tag="ex")
                        nc.scalar.activation(ex[:], s_ps[:], AF.Exp, scale=isq)
                        nc.vector.tensor_mul(ex[:], ex[:], cmask_sb[:, tk, :])
                        nc.tensor.matmul(out=sums_ps[:], lhsT=onescol_r[:],
                                         rhs=ex[:],
                                         start=(tk == 0), stop=(tk == 7))
                        nc.tensor.matmul(
                            out=av_ps[:],
                            lhsT=va[:, cb * MT + j, h * 64:(h + 1) * 64],
                            rhs=ex[:], start=(tk == 0), stop=(tk == 7))
                    rs = sc.tile([1, TC], F32, tag="rsum")
                    nc.vector.reciprocal(out=rs[:], in_=sums_ps[:])
                    rsr = sc.tile([1, TC], F32R, tag="rsumr")
                    nc.vector.tensor_copy(out=rsr[:], in_=rs[:])
                    rb_ps = pp.tile([128, TC], F32, tag="ps", space="PSUM")
                    nc.tensor.matmul(out=rb_ps[:], lhsT=ones1r_r[:],
                                     rhs=rsr[:], start=True, stop=True)
                    av_sb = sc.tile([64, TC], F32, tag="avs")
                    nc.vector.tensor_copy(out=av_sb[:], in_=av_ps[:])
                    nc.vector.tensor_mul(yT[off:off + 64, dk_h, :], av_sb[:], rb_ps[:64, :])

                for n0, nw in ((0, 512), (512, 256)):
                    wt = wp.tile([128, DK, 512], F32R, tag="w")
                    for dk in range(DK):
                        nc.sync.dma_start(out=wt[:, dk, :nw],
                                          in_=wop[l, dk * 128:(dk + 1) * 128, n0:n0 + nw])
                    for m in range(MT):
                        ps = pp.tile([128, 512], F32, tag="ps", space="PSUM")
                        for dk in range(DK):
                            nc.tensor.matmul(out=ps[:, :nw],
                                             lhsT=yT[:, dk, m * 128:(m + 1) * 128],
                                             rhs=wt[:, dk, :nw],
                                             start=(dk == 0), stop=(dk == DK - 1))
                        nc.vector.tensor_add(h_sb[:, m, n0:n0 + nw], ps[:, :nw],
                                             x_sb[:, m, n0:n0 + nw])

                f_sb = sb.tile([128, MT, D], F32, tag="f")
                rmsnorm_tiles(h_sb, f_sb, fnw[l], "fn")
                if stage == "attn" and l == 0:
                    nc.sync.dma_start(out=dbg["h"][:].rearrange("(m p) d -> p m d", p=128),
                                      in_=h_sb[:])
                    nc.sync.dma_start(out=dbg["f"][:].rearrange("(m p) d -> p m d", p=128),
                                      in_=f_sb[:])

                nc.sync.dma_start(out=f_in[:].rearrange("(m p) d -> p m d", p=128),
                                  in_=f_sb[:])
                nc.gpsimd.collective_compute("AllGather", ALU.bypass, replica_groups=GRP_ALL,
                                             ins=[f_in[:].opt()], outs=[f_out[:].opt()])

                # ================= gate + routing =================
                fT = sb.tile([128, DK, TC], F32, tag="dmaj")
                transpose_mtD(f_sb, fT)
                gw = sc.tile([128, DK, E], F32, tag="gw")
                for dk in range(DK):
                    nc.sync.dma_start(out=gw[:, dk, :],
                                      in_=gatew[l, dk * 128:(dk + 1) * 128, :])
                lg_sb = sc.tile([128, MT, E], F32, tag="lg")
                for m in range(MT):
                    ps = pp.tile([128, E], F32, tag="ps", space="PSUM")
                    for dk in range(DK):
                        nc.tensor.matmul(out=ps[:], lhsT=fT[:, dk, m * 128:(m + 1) * 128],
                                         rhs=gw[:, dk, :], start=(dk == 0), stop=(dk == DK - 1))
                    nc.vector.tensor_copy(out=lg_sb[:, m, :], in_=ps[:])
                nc.sync.dma_start(out=lg_in[:].rearrange("(m p) e -> p m e", p=128),
                                  in_=lg_sb[:])
                nc.gpsimd.collective_compute("AllGather", ALU.bypass, replica_groups=GRP_ALL,
                                             ins=[lg_in[:].opt()], outs=[lg_out[:].opt()])

                lga = sb.tile([128, 16, E], F32, tag="lga")
                nc.sync.dma_start(out=lga[:], in_=lg_out[:].rearrange("(a p) e -> p a e", p=128))
                zi = sc.tile([128, 1], I32, tag="zi")
                nc.vector.memset(zi[:], 0)
                nc.sync.dma_start(out=tmap[:][0:128, :], in_=zi[:])
                nc.sync.dma_start(out=tmap[:][128:256, :], in_=zi[:])
                nc.sync.dma_start(out=tmap[:][256:321, :], in_=zi[0:65, :])

                pm_ps = pp.tile([1, E], F32, tag="ps", space="PSUM")
                fr_ps = pp.tile([1, E], F32, tag="ps", space="PSUM")
                carry = sc.tile([1, E], F32, tag="carry")
                nc.vector.memset(carry[:], 0.0)
                z2cols = sb.tile([128, 16], F32, tag="z2")
                gidxf = sb.tile([128, 16], F32, tag="gidxf")
                keepf = sb.tile([128, 16], F32, tag="keepf")
                topc = sb.tile([128, 16], F32, tag="topc")
                rankc = sb.tile([128, 16], F32, tag="rankc")

                for i in range(16):
                    lg_i = lga[:, i, :]
                    rmax = sc.tile([128, 1], F32, tag="rmax")
                    nc.vector.tensor_reduce(rmax[:], lg_i, AX.X, ALU.max)
                    nrmax = sc.tile([128, 1], F32, tag="nrmax")
                    nc.vector.tensor_scalar_mul(nrmax[:], rmax[:], -1.0)
                    pe_e = sc.tile([128, E], F32, tag="pee")
                    ssum = sc.tile([128, 1], F32, tag="pss")
                    nc.scalar.activation(pe_e[:], lg_i, AF.Exp, bias=nrmax[:], accum_out=ssum[:])
                    rsum = sc.tile([128, 1], F32, tag="prs")
                    nc.vector.reciprocal(out=rsum[:], in_=ssum[:])
                    probs = sc.tile([128, E], F32, tag="prob")
                    nc.vector.tensor_scalar_mul(probs[:], pe_e[:], rsum[:])
                    zt = sc.tile([128, 1], F32, tag="zt")
                    nc.scalar.activation(zt[:], ssum[:], AF.Ln)
                    nc.vector.tensor_add(zt[:], zt[:], rmax[:])
                    nc.vector.tensor_mul(z2cols[:, i:i + 1], zt[:], zt[:])
                    mx = sc.tile([128, E], F32, tag="mx")
                    nc.vector.tensor_scalar(mx[:], lg_i, rmax[:], None, ALU.is_equal)
                    cand = sc.tile([128, E], F32, tag="cand")
                    nc.vector.tensor_mul(cand[:], mx[:], eiota_t[:])
                    nc.vector.scalar_tensor_tensor(cand[:], in0=mx[:], scalar=-100.0,
                                                   in1=cand[:], op0=ALU.mult, op1=ALU.add)
                    nc.vector.tensor_scalar_add(cand[:], cand[:], 100.0)
                    nc.vector.tensor_reduce(topc[:, i:i + 1], cand[:], AX.X, ALU.min)
                    onehot = sc.tile([128, E], F32, tag="oneh")
                    nc.vector.tensor_scalar(onehot[:], eiota_t[:], topc[:, i:i + 1],
                                            None, ALU.is_equal)
                    nc.tensor.matmul(out=pm_ps[:], lhsT=onescol_t[:], rhs=probs[:],
                                     start=(i == 0), stop=(i == 15))
                    nc.tensor.matmul(out=fr_ps[:], lhsT=onescol_t[:], rhs=onehot[:],
                                     start=(i == 0), stop=(i == 15))
                    rk_ps = pp.tile([128, E], F32, tag="ps", space="PSUM")
                    nc.tensor.matmul(out=rk_ps[:], lhsT=triu_t[:], rhs=onehot[:],
                                     start=True, stop=False)
                    nc.tensor.matmul(out=rk_ps[:], lhsT=ones1r_t[:], rhs=carry[:],
                                     start=False, stop=True)
                    selv = sc.tile([128, E], F32, tag="selv")
                    nc.vector.tensor_mul(selv[:], rk_ps[:], onehot[:])
                    nc.vector.tensor_reduce(rankc[:, i:i + 1], selv[:], AX.X, ALU.add)
                    cs_ps = pp.tile([1, E], F32, tag="ps", space="PSUM")
                    nc.tensor.matmul(out=cs_ps[:], lhsT=onescol_t[:], rhs=onehot[:],
                                     start=True, stop=True)
                    nc.vector.tensor_add(carry[:], carry[:], cs_ps[:])
                    nc.vector.tensor_scalar(keepf[:, i:i + 1], rankc[:, i:i + 1],
                                            float(CAP), None, ALU.is_lt)
                    gi = sc.tile([128, 1], F32, tag="gi")
                    nc.vector.tensor_scalar(gi[:], rankc[:, i:i + 1], float(CAP - 1),
                                            None, ALU.min)
                    nc.vector.scalar_tensor_tensor(gi[:], in0=topc[:, i:i + 1],
                                                   scalar=float(CAP), in1=gi[:],
                                                   op0=ALU.mult, op1=ALU.add)
                    nc.vector.tensor_copy(out=gidxf[:, i:i + 1], in_=gi[:])
                    mine = sc.tile([128, 1], F32, tag="mine")
                    nc.vector.tensor_scalar(mine[:], topc[:, i:i + 1], myexp_t[:],
                                            None, ALU.is_equal)
                    nc.vector.tensor_mul(mine[:], mine[:], keepf[:, i:i + 1])
                    slot = sc.tile([128, 1], F32, tag="slot")
                    nc.vector.tensor_scalar_add(slot[:], rankc[:, i:i + 1], float(-CAP))
                    nc.vector.tensor_mul(slot[:], slot[:], mine[:])
                    nc.vector.tensor_scalar_add(slot[:], slot[:], float(CAP))
                    slot_i = sc.tile([128, 1], I32, tag="sloti")
                    nc.vector.tensor_copy(out=slot_i[:], in_=slot[:])
                    nc.gpsimd.indirect_dma_start(
                        out=tmap[:],
                        out_offset=bass.IndirectOffsetOnAxis(ap=slot_i[:, 0:1], axis=0),
                        in_=tok_sb[:, i:i + 1], in_offset=None)

                pm_sb = sc.tile([1, E], F32, tag="pmsb")
                nc.vector.tensor_scalar_mul(pm_sb[:], pm_ps[:], 1.0 / N)
                fr_sb = sc.tile([1, E], F32, tag="frsb")
                nc.vector.tensor_scalar_mul(fr_sb[:], fr_ps[:], 1.0 / N)
                nc.vector.tensor_mul(pm_sb[:], pm_sb[:], fr_sb[:])
                bal = sc.tile([1, 1], F32, tag="bal")
                nc.vector.tensor_reduce(bal[:], pm_sb[:], AX.X, ALU.add)
                nc.vector.tensor_scalar_mul(bal[:], bal[:], AUX_W * E)
                nc.vector.tensor_add(aux_sb[:], aux_sb[:], bal[:])
                z2_ps = pp.tile([1, 16], F32, tag="ps", space="PSUM")
                nc.tensor.matmul(out=z2_ps[:], lhsT=onescol_t[:], rhs=z2cols[:],
                                 start=True, stop=True)
                zl = sc.tile([1, 1], F32, tag="zl")
                nc.vector.tensor_reduce(zl[:], z2_ps[:], AX.X, ALU.add)
                nc.vector.tensor_scalar_mul(zl[:], zl[:], Z_W / N)
                nc.vector.tensor_add(aux_sb[:], aux_sb[:], zl[:])

                # stage gidx/keep to DRAM for per-core combine gather
                nc.sync.dma_start(out=meta[:, 0:1].rearrange("(a p) one -> p (a one)", p=128),
                                  in_=gidxf[:])
                nc.sync.dma_start(out=meta[:, 1:2].rearrange("(a p) one -> p (a one)", p=128),
                                  in_=keepf[:])
                if stage == "route" and l == 0:
                    nc.sync.dma_start(out=dbg["top"][:].rearrange("(a p) one -> p (a one)", p=128),
                                      in_=topc[:])
                    nc.sync.dma_start(out=dbg["rank"][:].rearrange("(a p) one -> p (a one)", p=128),
                                      in_=rankc[:])

                # ================= dispatch + expert FFN =================
                tms = sc.tile([128, 3], I32, tag="tms")
                nc.sync.dma_start(out=tms[:, 0:1], in_=tmap[:][0:128, :])
                nc.sync.dma_start(out=tms[:, 1:2], in_=tmap[:][128:256, :])
                nc.sync.dma_start(out=tms[0:64, 2:3], in_=tmap[:][256:320, :])
                xe = sb.tile([128, 3, D], F32, tag="bigA")
                for s in range(3):
                    rows = 128 if s < 2 else 64
                    nc.gpsimd.indirect_dma_start(
                        out=xe[:rows, s, :], out_offset=None, in_=f_out[:],
                        in_offset=bass.IndirectOffsetOnAxis(ap=tms[:rows, s:s + 1], axis=0))
                if stage == "route" and l == 0:
                    nc.sync.dma_start(out=dbg["xe"][0:256, :].rearrange("(s p) d -> p s d", p=128),
                                      in_=xe[:, 0:2, :])
                    nc.sync.dma_start(out=dbg["xe"][256:320, :], in_=xe[0:64, 2, :])
                w13_l, w2_l, edt = (w13_0, w2_0, F32R) if l == 0 else (w13_1, w2_1, BF16)
                xeT = sb.tile([128, DK, CAP], F32R, tag="t6a")
                for s in range(3):
                    rows = 128 if s < 2 else 64
                    for dk in range(DK):
                        pe_t(xeT[:, dk, s * 128:s * 128 + rows],
                             xe[:rows, s, dk * 128:(dk + 1) * 128])
                if edt == BF16:
                    xeT16 = sb.tile([128, DK, CAP], BF16, tag="dmaj2")
                    nc.vector.tensor_copy(out=xeT16[:], in_=xeT[:])

                def xe_rhs(dk):
                    if edt == BF16:
                        return xeT16[:, dk, :]
                    return xeT[:, dk, :]

                actT = sb.tile([128, HK, CAP], edt if edt == BF16 else F32R, tag="bigB")
                for g in range(HK):
                    ps_g = pp.tile([128, CAP], F32, tag="ps", space="PSUM")
                    ps_u = pp.tile([128, CAP], F32, tag="ps", space="PSUM")
                    for half, psx in ((0, ps_g), (1, ps_u)):
                        mh = g + HK * half
                        wt = wp.tile([128, DK, 128], edt, tag="w")
                        for dk in range(DK):
                            nc.sync.dma_start(out=wt[:, dk, :],
                                              in_=w13_l[dk * 128:(dk + 1) * 128,
                                                        mh * 128:(mh + 1) * 128])
                        for dk in range(DK):
                            nc.tensor.matmul(out=psx[:], lhsT=wt[:, dk, :], rhs=xe_rhs(dk),
                                             start=(dk == 0), stop=(dk == DK - 1))
                    sg = sc.tile([128, CAP], F32, tag="sg")
                    nc.scalar.activation(sg[:], ps_g[:], AF.Silu)
                    nc.vector.tensor_mul(actT[:, g, :], sg[:], ps_u[:])

                def act_lhs(hk, s, rows):
                    a = actT[:, hk, s * 128:s * 128 + rows]
                    return a if edt == BF16 else a

                ye = sb.tile([128, 3, D], F32, tag="t6b")
                for n0, nw in ((0, 512), (512, 256)):
                    pss = []
                    for _psi in range(3):
                        ps_s = pp.tile([128, 512], F32, tag="ps", space="PSUM")
                        pss.append(ps_s)
                    for hk in range(HK):
                        wt = wp.tile([128, 512], edt, tag="w")
                        nc.sync.dma_start(out=wt[:, :nw],
                                          in_=w2_l[hk * 128:(hk + 1) * 128, n0:n0 + nw])
                        for s in range(3):
                            rows = 128 if s < 2 else 64
                            nc.tensor.matmul(out=pss[s][:rows, :nw], lhsT=act_lhs(hk, s, rows),
                                             rhs=wt[:, :nw],
                                             start=(hk == 0), stop=(hk == HK - 1))
                    for s in range(3):
                        rows = 128 if s < 2 else 64
                        nc.vector.tensor_copy(out=ye[:rows, s, n0:n0 + nw], in_=pss[s][:rows, :nw])
                if stage == "route" and l == 0:
                    nc.sync.dma_start(out=dbg["ye"][0:256, :].rearrange("(s p) d -> p s d", p=128),
                                      in_=ye[:, 0:2, :])
                    nc.sync.dma_start(out=dbg["ye"][256:320, :], in_=ye[0:64, 2, :])

                ye_in, ye_out, ydt = (ye_in0, ye_out0, F32) if l == 0 else (ye_in1, ye_out1, BF16)
                if ydt == BF16:
                    ye16 = sb.tile([128, 3, D], BF16, tag="t6c")
                    nc.vector.tensor_copy(out=ye16[:], in_=ye[:])
                    ye_src = ye16
                else:
                    ye_src = ye
                for s in range(3):
                    rows = 128 if s < 2 else 64
                    nc.sync.dma_start(out=ye_in[:][s * 128:s * 128 + rows, :],
                                      in_=ye_src[:rows, s, :])
                nc.gpsimd.collective_compute("AllGather", ALU.bypass, replica_groups=GRP_ALL,
                                             ins=[ye_in[:].opt()], outs=[ye_out[:].opt()])

                # ================= combine =================
                for m in range(MT):
                    mg = sc.tile([128, 2], F32, tag="mg")
                    nc.gpsimd.indirect_dma_start(
                        out=mg[:], out_offset=None, in_=meta[:],
                        in_offset=bass.IndirectOffsetOnAxis(ap=mytok_sb[:, m:m + 1], axis=0))
                    gidx_i = sc.tile([128, 1], I32, tag="gii")
                    nc.vector.tensor_copy(out=gidx_i[:], in_=mg[:, 0:1])
                    yg = sc.tile([128, D], ydt, tag="yg")
                    nc.gpsimd.indirect_dma_start(
                        out=yg[:], out_offset=None, in_=ye_out[:],
                        in_offset=bass.IndirectOffsetOnAxis(ap=gidx_i[:, 0:1], axis=0))
                    ygf = sc.tile([128, D], F32, tag="ygf")
                    if ydt == BF16:
                        nc.vector.tensor_copy(out=ygf[:], in_=yg[:])
                    else:
                        nc.vector.tensor_copy(out=ygf[:], in_=yg[:])
                    nc.vector.tensor_scalar_mul(ygf[:], ygf[:], mg[:, 1:2])
                    nc.vector.tensor_add(x_sb[:, m, :], h_sb[:, m, :], ygf[:])
                if stage == "layer" and l == 0:
                    nc.sync.dma_start(out=dbg["x1"][:].rearrange("(m p) d -> p m d", p=128),
                                      in_=x_sb[:])

            # ================= final norm + lm_head =================
            rmsnorm_tiles(x_sb, x_sb, lnf[:], "lf")
            xnT = sb.tile([128, DK, TC], F32, tag="dmaj")
            transpose_mtD(x_sb, xnT)
            xnT16 = sb.tile([128, DK, TC], BF16, tag="t6c")
            nc.vector.tensor_copy(out=xnT16[:], in_=xnT[:])
            nc.sync.dma_start(out=xt_in[:].rearrange("(p x) -> p x", p=128),
                              in_=xnT16[:].rearrange("p a x -> p (a x)"))
            nc.gpsimd.collective_compute("AllGather", ALU.bypass, replica_groups=GRP_ALL,
                                         ins=[xt_in[:].opt()], outs=[xt_out[:].opt()])
            xa = sb.tile([128, NC, DK, TC], BF16, tag="bigA")
            for cb in range(NC):
                nc.sync.dma_start(out=xa[:, cb, :, :].rearrange("p a x -> p (a x)"),
                                  in_=xt_out[cb].rearrange("(p x) -> p x", p=128))
            vch = [(i * 512, min(512, VP - i * 512)) for i in range((VP + 511) // 512)]
            for n0, nw in vch:
                wt = wp.tile([128, DK, 512], BF16, tag="w")
                for dk in range(DK):
                    nc.sync.dma_start(out=wt[:, dk, :nw],
                                      in_=wteT[dk * 128:(dk + 1) * 128, n0:n0 + nw])
                for mt in range(16):
                    cb, j = mt // 2, mt % 2
                    ps = pp.tile([128, 512], F32, tag="ps", space="PSUM")
                    for dk in range(DK):
                        nc.tensor.matmul(out=ps[:, :nw],
                                         lhsT=xa[:, cb, dk, j * 128:(j + 1) * 128],
                                         rhs=wt[:, dk, :nw],
                                         start=(dk == 0), stop=(dk == DK - 1))
                    ls = sc.tile([128, 512], F32, tag="ls")
                    nc.vector.tensor_copy(out=ls[:, :nw], in_=ps[:, :nw])
                    nc.sync.dma_start(out=logits_o[mt * 128:(mt + 1) * 128, n0:n0 + nw],
                                      in_=ls[:, :nw])
            nc.sync.dma_start(out=aux_o[:], in_=aux_sb[:])
    nc.compile()
    return nc


def prep_inputs(inputs):
    """Build the 8 per-core in_maps from the full model inputs."""
    f32 = np.float32
    idx = np.asarray(inputs["idx"]).reshape(-1)
    wte = np.asarray(inputs["wte"], f32)
    wqkv = np.ascontiguousarray(np.asarray(inputs["wqkv"], f32))
    o_proj = np.ascontiguousarray(np.asarray(inputs["o_proj"], f32))
    gate_w = np.ascontiguousarray(np.asarray(inputs["gate_w"], f32))
    w13 = np.asarray(inputs["w13"], f32)
    w2 = np.asarray(inputs["w2"], f32)
    import ml_dtypes
    bf = ml_dtypes.bfloat16

    freqs = 1.0 / (THETA ** (np.arange(0, HD, 2, dtype=f32) / HD))
    ang = np.outer(np.arange(T, dtype=f32), freqs).astype(f32)
    cos_full, sin_full = np.cos(ang).astype(f32), np.sin(ang).astype(f32)

    def bcast128(w):
        return np.broadcast_to(np.asarray(w, f32)[None, :], (128, w.shape[-1])).copy()

    qnw = np.stack([bcast128(np.asarray(inputs["q_norm_w"], f32)[l]) for l in range(L)])
    knw = np.stack([bcast128(np.asarray(inputs["k_norm_w"], f32)[l]) for l in range(L)])
    fnw = np.stack([bcast128(np.asarray(inputs["ffn_norm_w"], f32)[l]) for l in range(L)])
    lnf = bcast128(np.asarray(inputs["ln_f_w"], f32))

    ident = np.eye(128, dtype=f32)
    triu = np.triu(np.ones((128, 128), f32), 1)
    onescol = np.ones((128, 1), f32)
    ones1r = np.ones((1, 128), f32)
    eiota = np.broadcast_to(np.arange(E, dtype=f32)[None, :], (128, E)).copy()
    tokid_all = np.arange(N, dtype=np.int32)[:, None].copy()

    wteT_pad = np.zeros((D, NC * VP), bf)
    wteT_pad[:, :V] = wte.T.astype(bf)

    in_maps = []
    for c in range(NC):
        b, p0 = c // 4, (c % 4) * TC
        toks = idx[c * TC:(c + 1) * TC]
        cmask = (np.arange(1024)[:, None] <= (p0 + np.arange(TC))[None, :]).astype(f32)
        mytok = (c * TC + np.arange(TC, dtype=np.int32)).reshape(MT, 128).T.copy()
        in_maps.append(dict(
            x0=wte[toks].astype(f32),
            cos_t=cos_full[p0:p0 + TC], sin_t=sin_full[p0:p0 + TC],
            cmask=cmask, qnw=qnw, knw=knw, fnw=fnw, lnf=lnf,
            wqkv=wqkv, wop=o_proj, gatew=gate_w,
            w13_0=np.ascontiguousarray(w13[0, c]),
            w2_0=np.ascontiguousarray(w2[0, c]),
            w13_1=np.ascontiguousarray(w13[1, c]).astype(bf),
            w2_1=np.ascontiguousarray(w2[1, c]).astype(bf),
            wteT=np.ascontiguousarray(wteT_pad[:, c * VP:(c + 1) * VP]),
            ident=ident, triu=triu, onescol=onescol, ones1r=ones1r,
            eiota=eiota, myexp=np.full((128, 1), c, f32),
            tokid_all=tokid_all, mytok=mytok,
        ))
    return in_maps


_CACHED = {}


def run_kernel(inputs, stage="full"):
    if stage not in _CACHED:
        _CACHED[stage] = build_program(stage)
    nc = _CACHED[stage]
    in_maps = prep_inputs(inputs)
    res = run_bass_kernel_spmd(nc, in_maps, core_ids=list(range(NC)))
    return res


def kernel(**inputs):
    res = run_kernel(inputs, stage="full")
    logits = np.concatenate([res.results[c]["logits"] for c in range(NC)], axis=1)
    logits = logits[:, :V].reshape(B, T, V).astype(np.float32)
    aux = np.float32(res.results[0]["aux"][0, 0])
    return logits, aux


# revision 17
# speedup vs baseline: 33.4916x; 33.4916x over previous
"""Trainium2 Bass kernel for nn_LunarisCodex (2-layer Switch-MoE transformer).

8 NeuronCores, token-sharded trunk (256 tokens/core), expert-parallel MoE
(1 expert/core/layer), vocab-sharded lm_head (6283 cols/core).

Collectives per layer: AllGather of rope'd kT/v within batch groups
[[0-3],[4-7]]; global AllGathers of gate logits (tiny), ffn_in (dispatch),
and expert outputs (combine). Final: global AllGather of normed activations
(bf16, transposed) for the lm_head.

Routing (argmax, capacity ranks via strict-triangular matmuls, aux losses) is
computed replicated on every core from the gathered gate logits; dispatch and
combine are local indirect-DMA gathers keyed by device-computed index maps.

Precision: fp32r (full-rate ~13-bit-mantissa PE mode) for every matmul that
feeds routing (attention both layers, layer-0 experts); bf16 for layer-1
experts and lm_head; fp32 for gate logits, softmax, norms. This reproduces
the fp32 reference's expert assignments exactly (margin analysis: worst
perturbation/margin = 0.33).
"""

import math
import sys

import numpy as np

for _p in ("/opt/trn_rl_repo", "/root/.axon_site/_ro/trn_rl_repo"):
    if _p not in sys.path:
        sys.path.insert(0, _p)

import concourse.bacc as bacc
import concourse.bass as bass
import concourse.mybir as mybir
import concourse.tile as tile
from concourse.bass_utils import run_bass_kernel_spmd

F32 = mybir.dt.float32
F32R = mybir.dt.float32r
BF16 = mybir.dt.bfloat16
I32 = mybir.dt.int32
AF = mybir.ActivationFunctionType
ALU = mybir.AluOpType
AX = mybir.AxisListType

B, T, D, NH, HD = 2, 1024, 768, 12, 64
V, L, E, H = 50257, 2, 8, 2048
N = B * T
CAP = 320
AUX_W, Z_W, THETA, EPS = 0.01, 0.001, 10000.0, 1e-5
NC = 8
TC = N // NC          # 256 tokens/core
MT = TC // 128        # 2
DK = D // 128         # 6
HK = H // 128         # 16
VP = 6283             # padded vocab cols/core
GRP_BATCH = [[0, 1, 2, 3], [4, 5, 6, 7]]
GRP_ALL = [list(range(NC))]


def build_program(stage="full"):
    nc = bacc.Bacc(None, target_bir_lowering=False, num_devices=NC)

    def inp(name, shape, dt=F32):
        return nc.dram_tensor(name, shape, dt, kind="ExternalInput")

    x0 = inp("x0", [TC, D])
    cos_t = inp("cos_t", [TC, HD // 2])
    sin_t = inp("sin_t", [TC, HD // 2])
    cmask = inp("cmask", [1024, TC])
    qnw = inp("qnw", [L, 128, D])
    knw = inp("knw", [L, 128, D])
    fnw = inp("fnw", [L, 128, D])
    lnf = inp("lnf", [128, D])
    wqkv = inp("wqkv", [L, D, 3 * D], F32R)
    wop = inp("wop", [L, D, D], F32R)
    gatew = inp("gatew", [L, D, E])
    w13_0 = inp("w13_0", [D, 2 * H], F32R)
    w2_0 = inp("w2_0", [H, D], F32R)
    w13_1 = inp("w13_1", [D, 2 * H], BF16)
    w2_1 = inp("w2_1", [H, D], BF16)
    wteT = inp("wteT", [D, VP], BF16)
    ident = inp("ident", [128, 128])
    triu = inp("triu", [128, 128])
    onescol = inp("onescol", [128, 1])
    ones1r = inp("ones1r", [1, 128])
    eiota = inp("eiota", [128, E])
    myexp = inp("myexp", [128, 1])
    tokid_all = inp("tokid_all", [N, 1], I32)
    mytok = inp("mytok", [128, MT], I32)

    logits_o = nc.dram_tensor("logits", [N, VP], F32, kind="ExternalOutput")
    aux_o = nc.dram_tensor("aux", [1, 1], F32, kind="ExternalOutput")
    dbg = {}
    if stage != "full":
        for nm, shp in [("h", [TC, D]), ("f", [TC, D]), ("top", [N, 1]),
                        ("rank", [N, 1]), ("xe", [CAP, D]), ("ye", [CAP, D]),
                        ("x1", [TC, D])]:
            dbg[nm] = nc.dram_tensor("dbg_" + nm, shp, F32, kind="ExternalOutput")

    n_layers = 1 if stage in ("l1", "attn1") else L
    do_moe = stage != "attn1"
    do_lm = stage in ("full",)
    with tile.TileContext(nc) as tc:
        import os
        _scb = int(os.environ.get("K_SCB", "2"))
        _wpb = int(os.environ.get("K_WPB", "3"))
        with tc.tile_pool(name="sb", bufs=1) as sb, \
             tc.tile_pool(name="sc", bufs=_scb) as sc, \
             tc.tile_pool(name="wp", bufs=_wpb) as wp, \
             tc.tile_pool(name="fp", bufs=int(os.environ.get("K_FPB", "2"))) as fp, \
             tc.tile_pool(name="pp", bufs=8, space="PSUM") as pp, \
             tc.tile_pool(name="dram", bufs=1, space="DRAM") as dram:

            kv_in = dram.tile([2, D * TC], F32R)
            kv_out = dram.tile([4, 2, D * TC], F32R)
            lg_in = dram.tile([TC, E], F32)
            lg_outs = []
            for _li in range(L):
                lg_out_l = dram.tile([N, E], F32, addr_space="Shared")
                lg_outs.append(lg_out_l)
            f_in = dram.tile([TC, D], F32)
            f_outs = []
            for _li in range(L):
                f_out_l = dram.tile([N, D], F32, addr_space="Shared")
                f_outs.append(f_out_l)
            ye_in0 = dram.tile([CAP, D], F32)
            ye_out0 = dram.tile([NC * CAP, D], F32, addr_space="Shared")
            ye_in1 = dram.tile([CAP, D], BF16)
            ye_out1 = dram.tile([NC * CAP, D], BF16, addr_space="Shared")
            xt_in = dram.tile([D * TC], BF16)
            xt_out = dram.tile([NC, D * TC], BF16, addr_space="Shared")
            tmap = dram.tile([CAP + 1, 1], I32)
            meta = dram.tile([N, 2], F32)

            def const(name_tag, src_ap, shape, dt=F32):
                t = sb.tile(shape, dt, tag=name_tag)
                nc.sync.dma_start(out=t[:], in_=src_ap)
                return t

            ident_t = const("ident", ident[:], [128, 128])
            triu_t = const("triu", triu[:], [128, 128])
            onescol_t = const("onescol", onescol[:], [128, 1])
            ones1r_t = const("ones1r", ones1r[:], [1, 128])
            eiota_t = const("eiota", eiota[:], [128, E])
            myexp_t = const("myexp", myexp[:], [128, 1])
            cos_sb = const("cos", cos_t[:].rearrange("(m p) i -> p m i", p=128),
                           [128, MT, HD // 2])
            sin_sb = const("sin", sin_t[:].rearrange("(m p) i -> p m i", p=128),
                           [128, MT, HD // 2])
            cmask_sb = const("cmask", cmask[:].rearrange("(a p) q -> p a q", p=128),
                             [128, 8, TC])
            tok_sb = const("tok", tokid_all[:].rearrange("(a p) one -> p (a one)", p=128),
                           [128, 16], I32)
            mytok_sb = const("mytok", mytok[:], [128, MT], I32)
            onescol_r = sb.tile([128, 1], F32R, tag="onescolr")
            nc.vector.tensor_copy(out=onescol_r[:], in_=onescol_t[:])
            ones1r_r = sb.tile([1, 128], F32R, tag="ones1rr")
            nc.vector.tensor_copy(out=ones1r_r[:], in_=ones1r_t[:])

            eps_t = sb.tile([128, 1], F32, tag="eps")
            nc.vector.memset(eps_t[:], EPS)
            x_sb = sb.tile([128, MT, D], F32, tag="x")
            nc.sync.dma_start(out=x_sb[:], in_=x0[:].rearrange("(m p) d -> p m d", p=128))
            aux_sb = sb.tile([1, 1], F32, tag="aux")
            nc.vector.memset(aux_sb[:], 0.0)

            def pe_t(dst, src):
                """PE transpose one tile: src (p<=128, f<=128) -> dst (f, p)."""
                ps = pp.tile([128, 128], F32, tag="ps", space="PSUM")
                pn = src.partition_size()
                nc.tensor.transpose(out=ps[:src.shape[-1], :pn],
                                    in_=src, identity=ident_t[:pn, :pn])
                nc.vector.tensor_copy(out=dst, in_=ps[:dst.partition_size(), :dst.shape[-1]])

            def transpose_mtD(src, dst, mts=MT):
                for m in range(mts):
                    for dk in range(DK):
                        pe_t(dst[:, dk, m * 128:(m + 1) * 128],
                             src[:, m, dk * 128:(dk + 1) * 128])

            def rmsnorm_tiles(src, dst, w_dram, tg):
                w_t = sc.tile([128, D], F32, tag="nw")
                nc.sync.dma_start(out=w_t[:], in_=w_dram)
                for m in range(MT):
                    sq = sc.tile([128, D], F32, tag="nsq")
                    ssum = sc.tile([128, 1], F32, tag="nss")
                    nc.scalar.activation(sq[:], src[:, m, :], AF.Square, accum_out=ssum[:])
                    rms = sc.tile([128, 1], F32, tag="nrm")
                    nc.scalar.activation(rms[:], ssum[:], AF.Sqrt, bias=eps_t[:], scale=1.0 / D)
                    rinv = sc.tile([128, 1], F32, tag="nri")
                    nc.vector.reciprocal(out=rinv[:], in_=rms[:])
                    nc.vector.tensor_scalar_mul(dst[:, m, :], src[:, m, :], rinv[:])
                    nc.vector.tensor_mul(dst[:, m, :], dst[:, m, :], w_t[:])

            h_sb = sb.tile([128, MT, D], F32, tag="hres")

            for l in range(n_layers):
                lg_out = lg_outs[l]
                f_out = f_outs[l]
                # ================= attention =================
                xT = sb.tile([128, DK, TC], F32R, tag="dmaj")
                transpose_mtD(x_sb, xT)
                q_sb = sb.tile([128, MT, D], F32, tag="t6a")
                k_sb = sb.tile([128, MT, D], F32, tag="t6b")
                v_sb = sb.tile([128, MT, D], F32R, tag="t6c")
                qkv_dst = [(q_sb, 0), (k_sb, D), (v_sb, 2 * D)]
                nch = [(i * 512, min(512, 3 * D - i * 512)) for i in range((3 * D + 511) // 512)]
                for n0, nw in nch:
                    wt = wp.tile([128, DK, 512], F32R, tag="w")
                    for dk in range(DK):
                        nc.sync.dma_start(out=wt[:, dk, :nw],
                                          in_=wqkv[l, dk * 128:(dk + 1) * 128, n0:n0 + nw])
                    for m in range(MT):
                        ps = pp.tile([128, 512], F32, tag="ps", space="PSUM")
                        for dk in range(DK):
                            nc.tensor.matmul(out=ps[:, :nw],
                                             lhsT=xT[:, dk, m * 128:(m + 1) * 128],
                                             rhs=wt[:, dk, :nw],
                                             start=(dk == 0), stop=(dk == DK - 1))
                        for dst, base in qkv_dst:
                            lo, hi = max(n0, base), min(n0 + nw, base + D)
                            if lo < hi:
                                nc.vector.tensor_copy(out=dst[:, m, lo - base:hi - base],
                                                      in_=ps[:, lo - n0:hi - n0])
                rmsnorm_tiles(q_sb, q_sb, qnw[l], "qn")
                rmsnorm_tiles(k_sb, k_sb, knw[l], "kn")
                for src in (q_sb, k_sb):
                    for m in range(MT):
                        ap = src[:, m, :].rearrange("p (h i two) -> p h i two",
                                                    i=HD // 2, two=2)
                        e0, e1 = ap[:, :, :, 0], ap[:, :, :, 1]
                        cs = cos_sb[:, m, :].unsqueeze(1).broadcast_to([128, NH, HD // 2])
                        sn = sin_sb[:, m, :].unsqueeze(1).broadcast_to([128, NH, HD // 2])
                        t0 = sc.tile([128, NH, HD // 2], F32, tag="ra")
                        t1 = sc.tile([128, NH, HD // 2], F32, tag="rb")
                        t2 = sc.tile([128, NH, HD // 2], F32, tag="rc")
                        nc.vector.tensor_mul(t0[:], e0, cs)
                        nc.vector.tensor_mul(t1[:], e1, sn)
                        nc.vector.tensor_mul(t2[:], e0, sn)
                        nc.vector.tensor_mul(e1, e1, cs)
                        nc.vector.tensor_add(e1, e1, t2[:])
                        nc.vector.tensor_sub(e0, t0[:], t1[:])

                kT = sb.tile([128, DK, TC], F32R, tag="dmaj2")
                transpose_mtD(k_sb, kT)
                nc.sync.dma_start(out=kv_in[0].rearrange("(p x) -> p x", p=128),
                                  in_=kT[:].rearrange("p a x -> p (a x)"))
                nc.sync.dma_start(out=kv_in[1].rearrange("(p x) -> p x", p=128),
                                  in_=v_sb[:].rearrange("p m d -> p (m d)"))
                nc.gpsimd.collective_compute("AllGather", ALU.bypass,
                                             replica_groups=GRP_BATCH,
                                             ins=[kv_in[:].opt()], outs=[kv_out[:].opt()])
                kTa = sb.tile([128, 4 * DK, TC], F32R, tag="bigA")
                va = sb.tile([128, 4 * MT, D], F32R, tag="bigB")
                for cb in range(4):
                    nc.sync.dma_start(
                        out=kTa[:, cb * DK:(cb + 1) * DK, :].rearrange("p a x -> p (a x)"),
                        in_=kv_out[cb, 0].rearrange("(p x) -> p x", p=128))
                    nc.sync.dma_start(
                        out=va[:, cb * MT:(cb + 1) * MT, :].rearrange("p m d -> p (m d)"),
                        in_=kv_out[cb, 1].rearrange("(p x) -> p x", p=128))

                qT = sb.tile([128, DK, TC], F32R, tag="dmaj")
                transpose_mtD(q_sb, qT)

                yT = sb.tile([128, DK, TC], F32R, tag="yT")
                isq = 1.0 / math.sqrt(HD)
                for h in range(NH):
                    dk_h, off = h // 2, (h % 2) * 64
                    sums_ps = pp.tile([1, TC], F32, tag="ps", space="PSUM")
                    av_ps = pp.tile([64, TC], F32, tag="ps", space="PSUM")
                    for tk in range(8):
                        cb, j = tk // 2, tk % 2
                        s_ps = pp.tile([128, TC], F32, tag="ps", space="PSUM")
                        nc.tensor.matmul(
                            out=s_ps[:],
                            lhsT=kTa[off:off + 64, cb * DK + dk_h,
                                     j * 128:(j + 1) * 128],
                            rhs=qT[off:off + 64, dk_h, :],
                            start=True, stop=True)
                        ex = fp.tile([128, TC], F32R, tag="ex")
                        nc.scalar.activation(ex[:], s_ps[:], AF.Exp, scale=isq)
                        nc.vector.tensor_mul(ex[:], ex[:], cmask_sb[:, tk, :])
                        nc.tensor.matmul(out=sums_ps[:], lhsT=onescol_r[:],
                                         rhs=ex[:],
                                         start=(tk == 0), stop=(tk == 7))
                        nc.tensor.matmul(
                            out=av_ps[:],
                            lhsT=va[:, cb * MT + j, h * 64:(h + 1) * 64],
                            rhs=ex[:], start=(tk == 0), stop=(tk == 7))
                    rs = sc.tile([1, TC], F32, tag="rsum")
                    nc.vector.reciprocal(out=rs[:], in_=sums_ps[:])
                    rsr = sc.tile([1, TC], F32R, tag="rsumr")
                    nc.vector.tensor_copy(out=rsr[:], in_=rs[:])
                    rb_ps = pp.tile([128, TC], F32, tag="ps", space="PSUM")
                    nc.tensor.matmul(out=rb_ps[:], lhsT=ones1r_r[:],
                                     rhs=rsr[:], start=True, stop=True)
                    av_sb = sc.tile([64, TC], F32, tag="avs")
                    nc.vector.tensor_copy(out=av_sb[:], in_=av_ps[:])
                    nc.vector.tensor_mul(yT[off:off + 64, dk_h, :], av_sb[:], rb_ps[:64, :])

                for n0, nw in ((0, 512), (512, 256)):
                    wt = wp.tile([128, DK, 512], F32R, tag="w")
                    for dk in range(DK):
                        nc.sync.dma_start(out=wt[:, dk, :nw],
                                          in_=wop[l, dk * 128:(dk + 1) * 128, n0:n0 + nw])
                    for m in range(MT):
                        ps = pp.tile([128, 512], F32, tag="ps", space="PSUM")
                        for dk in range(DK):
                            nc.tensor.matmul(out=ps[:, :nw],
                                             lhsT=yT[:, dk, m * 128:(m + 1) * 128],
                                             rhs=wt[:, dk, :nw],
                                             start=(dk == 0), stop=(dk == DK - 1))
                        nc.vector.tensor_add(h_sb[:, m, n0:n0 + nw], ps[:, :nw],
                                             x_sb[:, m, n0:n0 + nw])

                f_sb = sb.tile([128, MT, D], F32, tag="f")
                rmsnorm_tiles(h_sb, f_sb, fnw[l], "fn")
                if stage == "attn" and l == 0:
                    nc.sync.dma_start(out=dbg["h"][:].rearrange("(m p) d -> p m d", p=128),
                                      in_=h_sb[:])
                    nc.sync.dma_start(out=dbg["f"][:].rearrange("(m p) d -> p m d", p=128),
                                      in_=f_sb[:])

                if not do_moe:
                    continue
                nc.sync.dma_start(out=f_in[:].rearrange("(m p) d -> p m d", p=128),
                                  in_=f_sb[:])
                nc.gpsimd.collective_compute("AllGather", ALU.bypass, replica_groups=GRP_ALL,
                                             ins=[f_in[:].opt()], outs=[f_out[:].opt()])

                # ================= gate + routing =================
                fT = sb.tile([128, DK, TC], F32, tag="dmaj")
                transpose_mtD(f_sb, fT)
                gw = sc.tile([128, DK, E], F32, tag="gw")
                for dk in range(DK):
                    nc.sync.dma_start(out=gw[:, dk, :],
                                      in_=gatew[l, dk * 128:(dk + 1) * 128, :])
                lg_sb = sc.tile([128, MT, E], F32, tag="lg")
                for m in range(MT):
                    ps = pp.tile([128, E], F32, tag="ps", space="PSUM")
                    for dk in range(DK):
                        nc.tensor.matmul(out=ps[:], lhsT=fT[:, dk, m * 128:(m + 1) * 128],
                                         rhs=gw[:, dk, :], start=(dk == 0), stop=(dk == DK - 1))
                    nc.vector.tensor_copy(out=lg_sb[:, m, :], in_=ps[:])
                nc.sync.dma_start(out=lg_in[:].rearrange("(m p) e -> p m e", p=128),
                                  in_=lg_sb[:])
                nc.gpsimd.collective_compute("AllGather", ALU.bypass, replica_groups=GRP_ALL,
                                             ins=[lg_in[:].opt()], outs=[lg_out[:].opt()])

                lga = sb.tile([128, 16, E], F32, tag="lga")
                nc.sync.dma_start(out=lga[:], in_=lg_out[:].rearrange("(a p) e -> p a e", p=128))
                zi = sc.tile([128, 1], I32, tag="zi")
                nc.vector.memset(zi[:], 0)
                nc.sync.dma_start(out=tmap[:][0:128, :], in_=zi[:])
                nc.sync.dma_start(out=tmap[:][128:256, :], in_=zi[:])
                nc.sync.dma_start(out=tmap[:][256:321, :], in_=zi[0:65, :])

                pm_ps = pp.tile([1, E], F32, tag="ps", space="PSUM")
                fr_ps = pp.tile([1, E], F32, tag="ps", space="PSUM")
                carry = sc.tile([1, E], F32, tag="carry")
                nc.vector.memset(carry[:], 0.0)
                z2cols = sb.tile([128, 16], F32, tag="z2")
                gidxf = sb.tile([128, 16], F32, tag="gidxf")
                keepf = sb.tile([128, 16], F32, tag="keepf")
                topc = sb.tile([128, 16], F32, tag="topc")
                rankc = sb.tile([128, 16], F32, tag="rankc")

                for i in range(16):
                    lg_i = lga[:, i, :]
                    rmax = sc.tile([128, 1], F32, tag="rmax")
                    nc.vector.tensor_reduce(rmax[:], lg_i, AX.X, ALU.max)
                    nrmax = sc.tile([128, 1], F32, tag="nrmax")
                    nc.vector.tensor_scalar_mul(nrmax[:], rmax[:], -1.0)
                    pe_e = sc.tile([128, E], F32, tag="pee")
                    ssum = sc.tile([128, 1], F32, tag="pss")
                    nc.scalar.activation(pe_e[:], lg_i, AF.Exp, bias=nrmax[:], accum_out=ssum[:])
                    rsum = sc.tile([128, 1], F32, tag="prs")
                    nc.vector.reciprocal(out=rsum[:], in_=ssum[:])
                    probs = sc.tile([128, E], F32, tag="prob")
                    nc.vector.tensor_scalar_mul(probs[:], pe_e[:], rsum[:])
                    zt = sc.tile([128, 1], F32, tag="zt")
                    nc.scalar.activation(zt[:], ssum[:], AF.Ln)
                    nc.vector.tensor_add(zt[:], zt[:], rmax[:])
                    nc.vector.tensor_mul(z2cols[:, i:i + 1], zt[:], zt[:])
                    mx = sc.tile([128, E], F32, tag="mx")
                    nc.vector.tensor_scalar(mx[:], lg_i, rmax[:], None, ALU.is_equal)
                    cand = sc.tile([128, E], F32, tag="cand")
                    nc.vector.tensor_mul(cand[:], mx[:], eiota_t[:])
                    nc.vector.scalar_tensor_tensor(cand[:], in0=mx[:], scalar=-100.0,
                                                   in1=cand[:], op0=ALU.mult, op1=ALU.add)
                    nc.vector.tensor_scalar_add(cand[:], cand[:], 100.0)
                    nc.vector.tensor_reduce(topc[:, i:i + 1], cand[:], AX.X, ALU.min)
                    onehot = sc.tile([128, E], F32, tag="oneh")
                    nc.vector.tensor_scalar(onehot[:], eiota_t[:], topc[:, i:i + 1],
                                            None, ALU.is_equal)
                    nc.tensor.matmul(out=pm_ps[:], lhsT=onescol_t[:], rhs=probs[:],
                                     start=(i == 0), stop=(i == 15))
                    nc.tensor.matmul(out=fr_ps[:], lhsT=onescol_t[:], rhs=onehot[:],
                                     start=(i == 0), stop=(i == 15))
                    rk_ps = pp.tile([128, E], F32, tag="ps", space="PSUM")
                    nc.tensor.matmul(out=rk_ps[:], lhsT=triu_t[:], rhs=onehot[:],
                                     start=True, stop=False)
                    nc.tensor.matmul(out=rk_ps[:], lhsT=ones1r_t[:], rhs=carry[:],
                                     start=False, stop=True)
                    selv = sc.tile([128, E], F32, tag="selv")
                    nc.vector.tensor_mul(selv[:], rk_ps[:], onehot[:])
                    nc.vector.tensor_reduce(rankc[:, i:i + 1], selv[:], AX.X, ALU.add)
                    cs_ps = pp.tile([1, E], F32, tag="ps", space="PSUM")
                    nc.tensor.matmul(out=cs_ps[:], lhsT=onescol_t[:], rhs=onehot[:],
                                     start=True, stop=True)
                    nc.vector.tensor_add(carry[:], carry[:], cs_ps[:])
                    nc.vector.tensor_scalar(keepf[:, i:i + 1], rankc[:, i:i + 1],
                                            float(CAP), None, ALU.is_lt)
                    gi = sc.tile([128, 1], F32, tag="gi")
                    nc.vector.tensor_scalar(gi[:], rankc[:, i:i + 1], float(CAP - 1),
                                            None, ALU.min)
                    nc.vector.scalar_tensor_tensor(gi[:], in0=topc[:, i:i + 1],
                                                   scalar=float(CAP), in1=gi[:],
                                                   op0=ALU.mult, op1=ALU.add)
                    nc.vector.tensor_copy(out=gidxf[:, i:i + 1], in_=gi[:])
                    mine = sc.tile([128, 1], F32, tag="mine")
                    nc.vector.tensor_scalar(mine[:], topc[:, i:i + 1], myexp_t[:],
                                            None, ALU.is_equal)
                    nc.vector.tensor_mul(mine[:], mine[:], keepf[:, i:i + 1])
                    slot = sc.tile([128, 1], F32, tag="slot")
                    nc.vector.tensor_scalar_add(slot[:], rankc[:, i:i + 1], float(-CAP))
                    nc.vector.tensor_mul(slot[:], slot[:], mine[:])
                    nc.vector.tensor_scalar_add(slot[:], slot[:], float(CAP))
                    slot_i = sc.tile([128, 1], I32, tag="sloti")
                    nc.vector.tensor_copy(out=slot_i[:], in_=slot[:])
                    nc.gpsimd.indirect_dma_start(
                        out=tmap[:],
                        out_offset=bass.IndirectOffsetOnAxis(ap=slot_i[:, 0:1], axis=0),
                        in_=tok_sb[:, i:i + 1], in_offset=None)

                pm_sb = sc.tile([1, E], F32, tag="pmsb")
                nc.vector.tensor_scalar_mul(pm_sb[:], pm_ps[:], 1.0 / N)
                fr_sb = sc.tile([1, E], F32, tag="frsb")
                nc.vector.tensor_scalar_mul(fr_sb[:], fr_ps[:], 1.0 / N)
                nc.vector.tensor_mul(pm_sb[:], pm_sb[:], fr_sb[:])
                bal = sc.tile([1, 1], F32, tag="bal")
                nc.vector.tensor_reduce(bal[:], pm_sb[:], AX.X, ALU.add)
                nc.vector.tensor_scalar_mul(bal[:], bal[:], AUX_W * E)
                nc.vector.tensor_add(aux_sb[:], aux_sb[:], bal[:])
                z2_ps = pp.tile([1, 16], F32, tag="ps", space="PSUM")
                nc.tensor.matmul(out=z2_ps[:], lhsT=onescol_t[:], rhs=z2cols[:],
                                 start=True, stop=True)
                zl = sc.tile([1, 1], F32, tag="zl")
                nc.vector.tensor_reduce(zl[:], z2_ps[:], AX.X, ALU.add)
                nc.vector.tensor_scalar_mul(zl[:], zl[:], Z_W / N)
                nc.vector.tensor_add(aux_sb[:], aux_sb[:], zl[:])

                # stage gidx/keep to DRAM for per-core combine gather
                nc.sync.dma_start(out=meta[:, 0:1].rearrange("(a p) one -> p (a one)", p=128),
                                  in_=gidxf[:])
                nc.sync.dma_start(out=meta[:, 1:2].rearrange("(a p) one -> p (a one)", p=128),
                                  in_=keepf[:])
                if stage == "route" and l == 0:
                    nc.sync.dma_start(out=dbg["top"][:].rearrange("(a p) one -> p (a one)", p=128),
                                      in_=topc[:])
                    nc.sync.dma_start(out=dbg["rank"][:].rearrange("(a p) one -> p (a one)", p=128),
                                      in_=rankc[:])

                # ================= dispatch + expert FFN =================
                tms = sc.tile([128, 3], I32, tag="tms")
                nc.sync.dma_start(out=tms[:, 0:1], in_=tmap[:][0:128, :])
                nc.sync.dma_start(out=tms[:, 1:2], in_=tmap[:][128:256, :])
                nc.sync.dma_start(out=tms[0:64, 2:3], in_=tmap[:][256:320, :])
                xe = sb.tile([128, 3, D], F32, tag="bigA")
                for s in range(3):
                    rows = 128 if s < 2 else 64
                    nc.gpsimd.indirect_dma_start(
                        out=xe[:rows, s, :], out_offset=None, in_=f_out[:],
                        in_offset=bass.IndirectOffsetOnAxis(ap=tms[:rows, s:s + 1], axis=0))
                if stage == "route" and l == 0:
                    nc.sync.dma_start(out=dbg["xe"][0:256, :].rearrange("(s p) d -> p s d", p=128),
                                      in_=xe[:, 0:2, :])
                    nc.sync.dma_start(out=dbg["xe"][256:320, :], in_=xe[0:64, 2, :])
                w13_l, w2_l, edt = (w13_0, w2_0, F32R) if l == 0 else (w13_1, w2_1, BF16)
                xeT = sb.tile([128, DK, CAP], F32R, tag="t6a")
                for s in range(3):
                    rows = 128 if s < 2 else 64
                    for dk in range(DK):
                        pe_t(xeT[:, dk, s * 128:s * 128 + rows],
                             xe[:rows, s, dk * 128:(dk + 1) * 128])
                if edt == BF16:
                    xeT16 = sb.tile([128, DK, CAP], BF16, tag="dmaj2")
                    nc.vector.tensor_copy(out=xeT16[:], in_=xeT[:])

                def xe_rhs(dk):
                    if edt == BF16:
                        return xeT16[:, dk, :]
                    return xeT[:, dk, :]

                actT = sb.tile([128, HK, CAP], edt if edt == BF16 else F32R, tag="bigB")
                for g in range(HK):
                    ps_g = pp.tile([128, CAP], F32, tag="ps", space="PSUM")
                    ps_u = pp.tile([128, CAP], F32, tag="ps", space="PSUM")
                    for half, psx in ((0, ps_g), (1, ps_u)):
                        mh = g + HK * half
                        wt = wp.tile([128, DK, 128], edt, tag="w")
                        for dk in range(DK):
                            nc.sync.dma_start(out=wt[:, dk, :],
                                              in_=w13_l[dk * 128:(dk + 1) * 128,
                                                        mh * 128:(mh + 1) * 128])
                        for dk in range(DK):
                            nc.tensor.matmul(out=psx[:], lhsT=wt[:, dk, :], rhs=xe_rhs(dk),
                                             start=(dk == 0), stop=(dk == DK - 1))
                    sg = fp.tile([128, CAP], F32, tag="sg")
                    nc.scalar.activation(sg[:], ps_g[:], AF.Silu)
                    nc.vector.tensor_mul(actT[:, g, :], sg[:], ps_u[:])

                def act_lhs(hk, s, rows):
                    a = actT[:, hk, s * 128:s * 128 + rows]
                    return a if edt == BF16 else a

                ye = sb.tile([128, 3, D], F32, tag="t6b")
                for n0, nw in ((0, 512), (512, 256)):
                    pss = []
                    for _psi in range(3):
                        ps_s = pp.tile([128, 512], F32, tag="ps", space="PSUM")
                        pss.append(ps_s)
                    for hk in range(HK):
                        wt = wp.tile([128, 512], edt, tag="w")
                        nc.sync.dma_start(out=wt[:, :nw],
                                          in_=w2_l[hk * 128:(hk + 1) * 128, n0:n0 + nw])
                        for s in range(3):
                            rows = 128 if s < 2 else 64
                            nc.tensor.matmul(out=pss[s][:rows, :nw], lhsT=act_lhs(hk, s, rows),
                                             rhs=wt[:, :nw],
                                             start=(hk == 0), stop=(hk == HK - 1))
                    for s in range(3):
                        rows = 128 if s < 2 else 64
                        nc.vector.tensor_copy(out=ye[:rows, s, n0:n0 + nw], in_=pss[s][:rows, :nw])
                if stage == "route" and l == 0:
                    nc.sync.dma_start(out=dbg["ye"][0:256, :].rearrange("(s p) d -> p s d", p=128),
                                      in_=ye[:, 0:2, :])
                    nc.sync.dma_start(out=dbg["ye"][256:320, :], in_=ye[0:64, 2, :])

                ye_in, ye_out, ydt = (ye_in0, ye_out0, F32) if l == 0 else (ye_in1, ye_out1, BF16)
                if ydt == BF16:
                    ye16 = sb.tile([128, 3, D], BF16, tag="t6c")
                    nc.vector.tensor_copy(out=ye16[:], in_=ye[:])
                    ye_src = ye16
                else:
                    ye_src = ye
                for s in range(3):
                    rows = 128 if s < 2 else 64
                    nc.sync.dma_start(out=ye_in[:][s * 128:s * 128 + rows, :],
                                      in_=ye_src[:rows, s, :])
                nc.gpsimd.collective_compute("AllGather", ALU.bypass, replica_groups=GRP_ALL,
                                             ins=[ye_in[:].opt()], outs=[ye_out[:].opt()])

                # ================= combine =================
                for m in range(MT):
                    mg = sc.tile([128, 2], F32, tag="mg")
                    nc.gpsimd.indirect_dma_start(
                        out=mg[:], out_offset=None, in_=meta[:],
                        in_offset=bass.IndirectOffsetOnAxis(ap=mytok_sb[:, m:m + 1], axis=0))
                    gidx_i = sc.tile([128, 1], I32, tag="gii")
                    nc.vector.tensor_copy(out=gidx_i[:], in_=mg[:, 0:1])
                    yg = sc.tile([128, D], ydt, tag="yg")
                    nc.gpsimd.indirect_dma_start(
                        out=yg[:], out_offset=None, in_=ye_out[:],
                        in_offset=bass.IndirectOffsetOnAxis(ap=gidx_i[:, 0:1], axis=0))
                    ygf = sc.tile([128, D], F32, tag="ygf")
                    if ydt == BF16:
                        nc.vector.tensor_copy(out=ygf[:], in_=yg[:])
                    else:
                        nc.vector.tensor_copy(out=ygf[:], in_=yg[:])
                    nc.vector.tensor_scalar_mul(ygf[:], ygf[:], mg[:, 1:2])
                    nc.vector.tensor_add(x_sb[:, m, :], h_sb[:, m, :], ygf[:])
                if stage == "layer" and l == 0:
                    nc.sync.dma_start(out=dbg["x1"][:].rearrange("(m p) d -> p m d", p=128),
                                      in_=x_sb[:])

            # ================= final norm + lm_head =================
            if not do_lm:
                nc.sync.dma_start(out=aux_o[:], in_=aux_sb[:])
                nc.sync.dma_start(out=logits_o[0:128, 0:128],
                                  in_=x_sb[:, 0, 0:128])
            rmsnorm_tiles(x_sb, x_sb, lnf[:], "lf")
            xnT = sb.tile([128, DK, TC], F32, tag="dmaj")
            transpose_mtD(x_sb, xnT)
            xnT16 = sb.tile([128, DK, TC], BF16, tag="t6c")
            nc.vector.tensor_copy(out=xnT16[:], in_=xnT[:])
            nc.sync.dma_start(out=xt_in[:].rearrange("(p x) -> p x", p=128),
                              in_=xnT16[:].rearrange("p a x -> p (a x)"))
            nc.gpsimd.collective_compute("AllGather", ALU.bypass, replica_groups=GRP_ALL,
                                         ins=[xt_in[:].opt()], outs=[xt_out[:].opt()])
            xa = sb.tile([128, NC, DK, TC], BF16, tag="bigA")
            for cb in range(NC):
                nc.sync.dma_start(out=xa[:, cb, :, :].rearrange("p a x -> p (a x)"),
                                  in_=xt_out[cb].rearrange("(p x) -> p x", p=128))
            vch = [(i * 512, min(512, VP - i * 512)) for i in range((VP + 511) // 512)]
            for n0, nw in vch:
                wt = wp.tile([128, DK, 512], BF16, tag="w")
                for dk in range(DK):
                    nc.sync.dma_start(out=wt[:, dk, :nw],
                                      in_=wteT[dk * 128:(dk + 1) * 128, n0:n0 + nw])
                for mt in range(16):
                    cb, j = mt // 2, mt % 2
                    ps = pp.tile([128, 512], F32, tag="ps", space="PSUM")
                    for dk in range(DK):
                        nc.tensor.matmul(out=ps[:, :nw],
                                         lhsT=xa[:, cb, dk, j * 128:(j + 1) * 128],
                                         rhs=wt[:, dk, :nw],
                                         start=(dk == 0), stop=(dk == DK - 1))
                    ls = sc.tile([128, 512], F32, tag="ls")
                    nc.vector.tensor_copy(out=ls[:, :nw], in_=ps[:, :nw])
                    nc.sync.dma_start(out=logits_o[mt * 128:(mt + 1) * 128, n0:n0 + nw],
                                      in_=ls[:, :nw])
            nc.sync.dma_start(out=aux_o[:], in_=aux_sb[:])
    nc.compile()
    return nc


def prep_inputs(inputs):
    """Build the 8 per-core in_maps from the full model inputs."""
    f32 = np.float32
    idx = np.asarray(inputs["idx"]).reshape(-1)
    wte = np.asarray(inputs["wte"], f32)
    wqkv = np.ascontiguousarray(np.asarray(inputs["wqkv"], f32))
    o_proj = np.ascontiguousarray(np.asarray(inputs["o_proj"], f32))
    gate_w = np.ascontiguousarray(np.asarray(inputs["gate_w"], f32))
    w13 = np.asarray(inputs["w13"], f32)
    w2 = np.asarray(inputs["w2"], f32)
    import ml_dtypes
    bf = ml_dtypes.bfloat16

    freqs = 1.0 / (THETA ** (np.arange(0, HD, 2, dtype=f32) / HD))
    ang = np.outer(np.arange(T, dtype=f32), freqs).astype(f32)
    cos_full, sin_full = np.cos(ang).astype(f32), np.sin(ang).astype(f32)

    def bcast128(w):
        return np.broadcast_to(np.asarray(w, f32)[None, :], (128, w.shape[-1])).copy()

    qnw = np.stack([bcast128(np.asarray(inputs["q_norm_w"], f32)[l]) for l in range(L)])
    knw = np.stack([bcast128(np.asarray(inputs["k_norm_w"], f32)[l]) for l in range(L)])
    fnw = np.stack([bcast128(np.asarray(inputs["ffn_norm_w"], f32)[l]) for l in range(L)])
    lnf = bcast128(np.asarray(inputs["ln_f_w"], f32))

    ident = np.eye(128, dtype=f32)
    triu = np.triu(np.ones((128, 128), f32), 1)
    onescol = np.ones((128, 1), f32)
    ones1r = np.ones((1, 128), f32)
    eiota = np.broadcast_to(np.arange(E, dtype=f32)[None, :], (128, E)).copy()
    tokid_all = np.arange(N, dtype=np.int32)[:, None].copy()

    wteT_pad = np.zeros((D, NC * VP), bf)
    wteT_pad[:, :V] = wte.T.astype(bf)

    in_maps = []
    for c in range(NC):
        b, p0 = c // 4, (c % 4) * TC
        toks = idx[c * TC:(c + 1) * TC]
        cmask = (np.arange(1024)[:, None] <= (p0 + np.arange(TC))[None, :]).astype(f32)
        mytok = (c * TC + np.arange(TC, dtype=np.int32)).reshape(MT, 128).T.copy()
        in_maps.append(dict(
            x0=wte[toks].astype(f32),
            cos_t=cos_full[p0:p0 + TC], sin_t=sin_full[p0:p0 + TC],
            cmask=cmask, qnw=qnw, knw=knw, fnw=fnw, lnf=lnf,
            wqkv=wqkv, wop=o_proj, gatew=gate_w,
            w13_0=np.ascontiguousarray(w13[0, c]),
            w2_0=np.ascontiguousarray(w2[0, c]),
            w13_1=np.ascontiguousarray(w13[1, c]).astype(bf),
            w2_1=np.ascontiguousarray(w2[1, c]).astype(bf),
            wteT=np.ascontiguousarray(wteT_pad[:, c * VP:(c + 1) * VP]),
            ident=ident, triu=triu, onescol=onescol, ones1r=ones1r,
            eiota=eiota, myexp=np.full((128, 1), c, f32),
            tokid_all=tokid_all, mytok=mytok,
        ))
    return in_maps


_CACHED = {}


def run_kernel(inputs, stage="full"):
    if stage not in _CACHED:
        _CACHED[stage] = build_program(stage)
    nc = _CACHED[stage]
    in_maps = prep_inputs(inputs)
    res = run_bass_kernel_spmd(nc, in_maps, core_ids=list(range(NC)))
    return res


def kernel(**inputs):
    res = run_kernel(inputs, stage="full")
    logits = np.concatenate([res.results[c]["logits"] for c in range(NC)], axis=1)
    logits = logits[:, :V].reshape(B, T, V).astype(np.float32)
    aux = np.float32(res.results[0]["aux"][0, 0])
    return logits, aux
